# revision 17
# baseline (speedup 1.0000x reference)
"""DETR-style matching loss on 8 Trainium2 NeuronCores.

Device (data-parallel over batch, 8 samples/core): the [B,P,T] pairwise
cost matrix  cost = NLL + mask*(5*L1 + 2*GIoU_loss)  computed per core as
a [1200, 150] slab (rows = (b,p) packed, 10 tiles of 128 partitions).
Host: scipy Hungarian per sample on the device cost + the final
gather/mean (inherently sequential scalar work on 150x150 matrices).

v4 design:
- All per-(row,t) bilinear terms come from one bf16 3-pass PE matmul per
  PSUM bank (exact to ~2^-25): label-quantity broadcasts AND pairwise
  pre-sums (areal+apE, lwE+pwE, lhE+phE) AND pairwise pre-diffs for L1.
  Per bank: K=15 (6 selector rows = 2 segments x hi/mid/lo + up to 3
  pred-side values x hi/mid/lo against slot-indicator rows), N=450.
- NLL + 4*mask from an fp32 one-hot matmul (exact: one product/output).
- The nonlinear chain runs as fused custom-DVE ops + stock tensor ops
  balanced across DVE/GpSimd/ScalarE.
- DMA is batched: 3 whole-kernel input DMAs (pt/rhs/scal, b or tile
  packed into a middle axis), 1 bc DMA per tile, and cost/nll
  accumulated in SBUF slabs shipped out in 3 chunked DMAs each.

Math (per pair, r=(b,p) row, t):
  iw = min(px1,lx1) - max(px0,lx0);  ih likewise;  inter = relu.iw*relu.ih
  ue = (areal + area_p + EPS) - inter          [= union + EPS]
  ew = (lwE + pwE) - iw;  eh = (lhE + phE) - ih   [min+max=sum identity]
  ee = ew*eh + EPS                              [= enclose + EPS]
  rc = 1/(ue*ee)   [approx recip + 1 Newton step]
  cost = [nll + 4*mask] + 2*mask*( 0.625*sum|d| - (inter*ee + ue^2)*rc )
"""

import sys

import numpy as np

for _p in ("/opt/trn_rl_repo",):
    if _p not in sys.path:
        sys.path.append(_p)

import ml_dtypes

import concourse.bacc as bacc
import concourse.tile as tile
from concourse import mybir
from concourse.bass_utils import run_bass_kernel_spmd

F32 = mybir.dt.float32
BF16 = mybir.dt.bfloat16
NPBF = ml_dtypes.bfloat16
EPS = np.float32(1e-7)
IMG_SIZE = np.float32(320.0)
NO_OBJ = 80

B, P, T, C = 64, 150, 150, 81
NCORES = 8
BL = B // NCORES          # 8 samples per core
ROWS = BL * P             # 1200 (b,p) rows per core
NTILES = (ROWS + 127) // 128   # 10
NK = 15                   # bc-matmul contraction rows per bank
KN = 96                   # nll contraction rows (81 lnp + 15 ones-pad)

# bc slot layout: slot q of bank j=q//3 at psum col 512*j + 150*(q%3).
# bank0 stays PSUM-resident; banks 1-3 are copied to SBUF.
SLOT = {"lx1": 0, "ly1": 1, "dcx": 2,
        "lx0": 3, "ly0": 4, "dcy": 5,
        "dw": 6, "dh": 7, "mask2": 8,
        "uepre": 9, "ewpre": 10, "ehpre": 11}
# slot -> pred-side presum value name (None = no pred part)
PRESUM_OF = {"dcx": "npcx", "dcy": "npcy", "dw": "npw", "dh": "nph",
             "uepre": "apE", "ewpre": "pwE", "ehpre": "phE"}


def _tile_segments():
    tiles = []
    for k in range(NTILES):
        r0, r1 = k * 128, min(k * 128 + 128, ROWS)
        segs, r = [], r0
        while r < r1:
            b = r // P
            p0 = r % P
            ln = min(r1 - r, P - p0)
            segs.append((b, p0, ln, r - r0))
            r += ln
        tiles.append(segs)
    return tiles


TILES = _tile_segments()
# flat segment list for the nll slab pages
SEGS = [(k, si, b, p0, ln, off)
        for k, segs in enumerate(TILES)
        for si, (b, p0, ln, off) in enumerate(segs)]
NSEG = len(SEGS)   # 17

_CACHE = {}
TRACE = False
LAST_EXEC_NS = None
LAST_RESULTS = None


def _register_custom_ops():
    if "ops" in _CACHE:
        return _CACHE["ops"]
    from concourse import dve_ops
    from concourse.dve_spec import (
        Spec, Src0, Src1, C0, C1, C2, Zero, relu, sq, maxx, minn, lower,
        _has_src1,
    )
    from concourse.dve_uop import DveOpSpec

    def ref_iw(in0, in1, s0, s1, imm2):
        return np.minimum(in0, s0) - np.maximum(in1, s1)

    def ref_relumul(in0, in1, s0, s1, imm2):
        return np.maximum(in0, 0) * np.maximum(in1, 0)

    def ref_fmaeps(in0, in1, s0, s1, imm2):
        return in0 * in1 + imm2

    def ref_addsq(in0, in1, s0, s1, imm2):
        return in0 + in1 * in1

    def ref_abs2(in0, in1, s0, s1, imm2):
        return (np.abs(in0) + np.abs(in1)) * imm2

    specs = {
        "DETR_IW": Spec(body=minn(Src0, C0) - maxx(Src1, C1),
                        reference=ref_iw),
        "DETR_RELUMUL": Spec(body=relu(Src0) * relu(Src1),
                             reference=ref_relumul),
        "DETR_FMAEPS": Spec(body=Src0 * Src1 + C2, reference=ref_fmaeps),
        "DETR_ADDSQ": Spec(body=Src0 + sq(Src1), reference=ref_addsq),
        "DETR_ABS2": Spec(
            body=(maxx(Src0, Zero - Src0) + maxx(Src1, Zero - Src1)) * C2,
            reference=ref_abs2),
    }
    ops = {}
    existing = {op.name for op in dve_ops.OPS}
    for name, spec in specs.items():
        if name in existing:
            ops[name] = next(o for o in dve_ops.OPS if o.name == name)
            continue
        shas = {}
        for ver in ("v3", "v4"):
            try:
                uops = lower(spec, ver=ver)
                s = DveOpSpec(name=name, opcode=1, uops=uops,
                              rd1_en=_has_src1(spec))
                shas[ver] = s.sha(ver)
            except Exception:
                pass
        op = dve_ops.DveOp(name, spec, subdim=False, uops_sha=shas)
        dve_ops.OPS.append(op)
        dve_ops.CUSTOM_DVE_SPECS[name] = spec
        dve_ops._SUB_OPCODE_FOR_NAME[name] = (
            max(dve_ops._SUB_OPCODE_FOR_NAME.values()) + 1)
        ops[name] = op
    _CACHE["ops"] = ops
    return ops


def _build_program():
    ops = _register_custom_ops()
    nc = bacc.Bacc(None, target_bir_lowering=False)

    # batched inputs: b (or tile) packed into a middle axis for few, fat DMAs
    pt_d = nc.declare_dram_parameter("pt", [C, BL, P], F32, isOutput=False)
    rhs_d = nc.declare_dram_parameter("rhsnm", [KN, BL, T], F32,
                                      isOutput=False)
    scal_d = nc.declare_dram_parameter("scal", [128, NTILES, 4], F32,
                                       isOutput=False)
    bcr_d = nc.declare_dram_parameter("bcr", [NTILES, NK, 2312], BF16,
                                      isOutput=False)
    cost_d = nc.declare_dram_parameter("cost", [128, NTILES, T], F32,
                                       isOutput=True)
    nll_d = nc.declare_dram_parameter("nllm", [128, NSEG, T], F32,
                                      isOutput=True)

    from contextlib import ExitStack

    with tile.TileContext(nc) as tc, ExitStack() as ctx:
        consts = ctx.enter_context(tc.tile_pool(name="consts", bufs=1))
        perb = ctx.enter_context(tc.tile_pool(name="perb", bufs=3))
        pert = ctx.enter_context(tc.tile_pool(name="pert", bufs=3))
        bcs = ctx.enter_context(tc.tile_pool(name="bcs", bufs=3))
        work = ctx.enter_context(tc.tile_pool(name="work", bufs=3))
        slab = ctx.enter_context(tc.tile_pool(name="slab", bufs=1))
        bcp0 = ctx.enter_context(tc.tile_pool(name="bcp0", bufs=2,
                                              space="PSUM"))
        bcp = ctx.enter_context(tc.tile_pool(name="bcp", bufs=1, space="PSUM"))
        nll0p = ctx.enter_context(tc.tile_pool(name="nll0p", bufs=2,
                                               space="PSUM"))
        nll1p = ctx.enter_context(tc.tile_pool(name="nll1p", bufs=1,
                                               space="PSUM"))

        v = nc.vector
        g = nc.gpsimd
        s = nc.scalar

        # whole-kernel batched inputs
        ptall = consts.tile([C, BL, P], F32)
        nc.sync.dma_start(out=ptall[:], in_=pt_d[:])
        rsall = consts.tile([KN, BL, T], F32)
        nc.sync.dma_start(out=rsall[:], in_=rhs_d[:])
        scall = consts.tile([128, NTILES, 4], F32)
        nc.sync.dma_start(out=scall[:], in_=scal_d[:])

        # output slabs
        costall = slab.tile([128, NTILES, T], F32)
        nllall = slab.tile([128, NSEG, T], F32)

        lhs = {}

        def ensure_b(b):
            if b in lhs:
                return
            lt = perb.tile([KN, P], F32, tag="lhs")
            nc.vector.memset(lt[64:KN, :], 1.0)
            nc.scalar.activation(out=lt[0:C, :], in_=ptall[:, b, :],
                                 func=mybir.ActivationFunctionType.Ln)
            lhs[b] = lt

        segidx = 0
        for k, segs in enumerate(TILES):
            nrow = sum(sg[2] for sg in segs)
            for b, _, _, _ in segs:
                ensure_b(b)

            # ---- bc matmuls: one per PSUM bank, K=15 bf16, N=450 ----
            bt = pert.tile([NK, 2312], BF16, tag="bt")
            nc.sync.dma_start(out=bt[:], in_=bcr_d[k])
            ps0 = bcp0.tile([128, 512], F32, tag="bcps0")
            ps = bcp.tile([128, 1536], F32, tag="bcps")
            for j in range(4):
                out = (ps0[:, 0:450] if j == 0
                       else ps[:, 512 * (j - 1):512 * (j - 1) + 450])
                nc.tensor.matmul(out,
                                 bt[:, 1800 + 128 * j:1800 + 128 * (j + 1)],
                                 bt[:, 450 * j:450 * (j + 1)],
                                 start=True, stop=True)

            # copy banks 1-3 to SBUF (split DVE / ACT) — frees ps for k+1
            sb = bcs.tile([128, 1536], F32, tag="bcsb")
            v.tensor_copy(sb[:, 0:512], ps[:, 0:512])
            s.copy(sb[:, 512:1536], ps[:, 512:1536])

            def bcP(name):
                off = 150 * (SLOT[name] % 3)
                return ps0[0:nrow, off:off + T]

            def bcS(name):
                q = SLOT[name]
                off = 512 * (q // 3 - 1) + 150 * (q % 3)
                return sb[0:nrow, off:off + T]

            # ---- nll + 4*mask (fp32 one-hot matmul per segment) ----
            for si, (b, p0, ln, off) in enumerate(segs):
                pool_ = nll0p if si == 0 else nll1p
                nm = pool_.tile([128, 512], F32, tag=f"nll{si}",
                                name=f"nll{si}")
                nc.tensor.matmul(nm[0:ln, 0:T], lhs[b][:, p0:p0 + ln],
                                 rsall[:, b, :], start=True, stop=True)
                s.copy(out=nllall[0:ln, segidx, :], in_=nm[0:ln, 0:T])
                segidx += 1

            # ---- pred xyxy scalars ----
            px1, px0, py1, py0 = (scall[0:nrow, k, i:i + 1] for i in range(4))

            def wt(tag, w=T):
                return work.tile([128, w], F32, tag=tag, name=tag)

            # ---- fused chain ----
            t3 = wt("t3", 3 * T)
            iw = t3[0:nrow, T:2 * T]
            ih = t3[0:nrow, 2 * T:3 * T]
            inter = t3[0:nrow, 0:T]
            v._custom_dve(ops["DETR_IW"], out=iw, in0=bcP("lx1"),
                          in1=bcS("lx0"), s0=px1, s1=px0)
            v._custom_dve(ops["DETR_IW"], out=ih, in0=bcP("ly1"),
                          in1=bcS("ly0"), s0=py1, s1=py0)
            v._custom_dve(ops["DETR_RELUMUL"], out=inter, in0=iw, in1=ih)

            # [ue|ew|eh] = [uepre|ewpre|ehpre] - [inter|iw|ih]  (one GPS op)
            uew = wt("uew", 3 * T)
            g.tensor_sub(uew[0:nrow, :], sb[0:nrow, 1024:1024 + 3 * T],
                         t3[0:nrow, :])
            ue = uew[0:nrow, 0:T]
            ew = uew[0:nrow, T:2 * T]
            eh = uew[0:nrow, 2 * T:3 * T]

            ee = wt("ee")[0:nrow, :]
            v._custom_dve(ops["DETR_FMAEPS"], out=ee, in0=ew, in1=eh,
                          imm2=float(EPS))
            prod = wt("prod")[0:nrow, :]
            v.tensor_mul(prod, ue, ee)
            t1 = wt("t1")[0:nrow, :]
            v.tensor_mul(t1, inter, ee)
            r0 = wt("r0")[0:nrow, :]
            rc = wt("rc")[0:nrow, :]
            v.reciprocal_approx_accurate(out=rc, in_=prod, scratch=r0)
            gnum = wt("gnum")[0:nrow, :]
            v._custom_dve(ops["DETR_ADDSQ"], out=gnum, in0=t1, in1=ue)
            gterm = wt("gterm")[0:nrow, :]
            g.tensor_mul(gterm, gnum, rc)

            s01 = wt("s01")[0:nrow, :]
            v._custom_dve(ops["DETR_ABS2"], out=s01, in0=bcP("dcx"),
                          in1=bcS("dcy"), imm2=0.625)
            s23 = wt("s23")[0:nrow, :]
            v._custom_dve(ops["DETR_ABS2"], out=s23, in0=bcS("dw"),
                          in1=bcS("dh"), imm2=0.625)
            l1t = wt("l1t")[0:nrow, :]
            g.tensor_add(l1t, s01, s23)
            pre = wt("pre")[0:nrow, :]
            g.tensor_sub(pre, l1t, gterm)
            g.tensor_mul(costall[0:nrow, k, :], pre, bcS("mask2"))

            # chunked slab flushes
            if k in (3, 6, 9):
                lo = {3: 0, 6: 4, 9: 7}[k]
                nc.sync.dma_start(out=cost_d[:, lo:k + 1, :],
                                  in_=costall[:, lo:k + 1, :])
                slo = {3: 0, 6: 7, 9: 12}[k]
                shi = segidx
                nc.sync.dma_start(out=nll_d[:, slo:shi, :],
                                  in_=nllall[:, slo:shi, :])

    nc.finalize()
    return nc


def _decomp3(vv):
    """f32 -> three bf16 arrays summing to v (error ~2^-25 |v|)."""
    vv = np.asarray(vv, np.float32)
    h = vv.astype(NPBF)
    r = vv - h.astype(np.float32)
    m = r.astype(NPBF)
    lo = (r - m.astype(np.float32)).astype(NPBF)
    return h, m, lo


def _host_prep(prob_class, predict_bbox, labels):
    pc = np.asarray(prob_class, np.float32)
    pb = np.asarray(predict_bbox, np.float32)
    lab = np.asarray(labels, np.float32)

    lb = lab[..., :4] / IMG_SIZE
    cls = lab[..., 4].astype(np.int32)
    mask = (cls != NO_OBJ).astype(np.float32)

    lcx, lcy, lw, lh = (lb[..., i] for i in range(4))
    half = np.float32(0.5)
    lx0 = lcx - half * lw
    lx1 = lcx + half * lw
    ly0 = lcy - half * lh
    ly1 = lcy + half * lh
    lwE = lx1 - lx0
    lhE = ly1 - ly0
    areal = lwE * lhE

    labvals = {"lx1": lx1, "ly1": ly1, "dcx": lcx,
               "lx0": lx0, "ly0": ly0, "dcy": lcy,
               "dw": lw, "dh": lh, "mask2": 2.0 * mask,
               "uepre": areal, "ewpre": lwE, "ehpre": lhE}
    qlab = np.zeros((B, 1800), np.float32)
    for name, q in SLOT.items():
        off = 450 * (q // 3) + 150 * (q % 3)
        qlab[:, off:off + T] = labvals[name]

    pcx, pcy, pw, ph = (pb[..., i] for i in range(4))
    px0 = pcx - half * pw
    px1 = pcx + half * pw
    py0 = pcy - half * ph
    py1 = pcy + half * ph
    pwE = px1 - px0
    phE = py1 - py0
    apE = pwE * phE + EPS
    presvals = {"npcx": -pcx, "npcy": -pcy, "npw": -pw, "nph": -ph,
                "apE": apE, "pwE": pwE, "phE": phE}   # [B, P] each

    # scal packed [128, NTILES, 4] per core (row r of tile k = global row
    # 128k + r)
    scal_rows = np.stack([px1, px0, py1, py0], axis=-1).reshape(
        NCORES, ROWS, 4)
    scal = np.zeros((NCORES, 128, NTILES, 4), np.float32)
    for k in range(NTILES):
        r0, r1 = k * 128, min(k * 128 + 128, ROWS)
        scal[:, 0:r1 - r0, k, :] = scal_rows[:, r0:r1, :]

    # nll rhs [KN, B-local, T] per core
    rhsnm = np.zeros((B, KN, T), np.float32)
    bi = np.repeat(np.arange(B), T)
    ti = np.tile(np.arange(T), B)
    rhsnm[bi, cls.ravel(), ti] = np.float32(-1.0)
    rhsnm[:, 81, :] = np.float32(4.0) * mask
    rhs_t = np.ascontiguousarray(
        rhsnm.reshape(NCORES, BL, KN, T).transpose(0, 2, 1, 3))

    # ln arg [C, B-local, P] per core
    pt_t = np.ascontiguousarray(
        pc.reshape(NCORES, BL, P, C).transpose(0, 3, 1, 2))

    qlab3 = _decomp3(qlab)                       # 3 x [B, 1800]
    pres3 = {n: _decomp3(val) for n, val in presvals.items()}

    bcr = np.zeros((NCORES, NTILES, NK, 2312), NPBF)
    for k, segs in enumerate(TILES):
        for core in range(NCORES):
            for si, (b, p0, ln, off) in enumerate(segs):
                gb = core * BL + b
                for p3 in range(3):
                    # label rows: values on rhs cols, sel on lhsT cols
                    bcr[core, k, 3 * si + p3, 0:1800] = qlab3[p3][gb]
                    for j in range(4):
                        bcr[core, k, 3 * si + p3,
                            1800 + 128 * j + off:1800 + 128 * j + off + ln] \
                            = NPBF(1.0)
            # presum rows 6..14: per-bank slot m
            for name, q in SLOT.items():
                pn = PRESUM_OF.get(name)
                if pn is None:
                    continue
                j, m = q // 3, q % 3
                coff = 450 * j + 150 * m
                for si, (b, p0, ln, off) in enumerate(segs):
                    gb = core * BL + b
                    for p3 in range(3):
                        row = 6 + 3 * m + p3
                        bcr[core, k, row,
                            1800 + 128 * j + off:1800 + 128 * j + off + ln] \
                            = pres3[pn][p3][gb, p0:p0 + ln]
                        bcr[core, k, row, coff:coff + T] = NPBF(1.0)

    in_maps = []
    for core in range(NCORES):
        in_maps.append({
            "pt": pt_t[core],
            "rhsnm": rhs_t[core],
            "bcr": bcr[core],
            "scal": scal[core],
        })
    return in_maps


def _hungarian_np(cost):
    """Jonker-Volgenant LSA fallback (same algorithm as scipy)."""
    cost = np.asarray(cost, dtype=np.float64)
    n, m = cost.shape
    INF = float("inf")
    u = np.zeros(n + 1)
    vv = np.zeros(m + 1)
    p = np.zeros(m + 1, dtype=np.int64)
    way = np.zeros(m + 1, dtype=np.int64)
    for i in range(1, n + 1):
        p[0] = i
        j0 = 0
        minv = np.full(m + 1, INF)
        used = np.zeros(m + 1, dtype=bool)
        while True:
            used[j0] = True
            i0 = p[j0]
            free = ~used[1:]
            cur = cost[i0 - 1] - u[i0] - vv[1:]
            better = free & (cur < minv[1:])
            minv[1:] = np.where(better, cur, minv[1:])
            way[1:] = np.where(better, j0, way[1:])
            masked = np.where(free, minv[1:], INF)
            j1 = int(np.argmin(masked)) + 1
            delta = masked[j1 - 1]
            uj = np.nonzero(used)[0]
            u[p[uj]] += delta
            vv[uj] -= delta
            minv[1:] = np.where(free, minv[1:] - delta, minv[1:])
            j0 = j1
            if p[j0] == 0:
                break
        while j0 != 0:
            j1 = way[j0]
            p[j0] = p[j1]
            j0 = j1
    cols = np.nonzero(p[1:])[0]
    rows = p[1:][cols] - 1
    order = np.argsort(rows)
    return rows[order], cols[order]


def _lsa(cost):
    try:
        from scipy.optimize import linear_sum_assignment
        return linear_sum_assignment(cost)
    except Exception:
        return _hungarian_np(cost)


def _install_profile_shim():
    import types

    if "antenv.axon_hooks" not in sys.modules:
        import antenv

        mod = types.ModuleType("antenv.axon_hooks")
        mod._HOOK = None
        mod.set_axon_ntff_profile_hook = lambda h: setattr(mod, "_HOOK", h)
        mod.get_axon_ntff_profile_hook = lambda: mod._HOOK
        sys.modules["antenv.axon_hooks"] = mod
        antenv.axon_hooks = mod
    import antenv.axon_hooks as ah

    if ah.get_axon_ntff_profile_hook() is None:
        try:
            from trn_agent_boot.trn_boot import _ntff_profile_via_ctypes

            hook = _ntff_profile_via_ctypes("/opt/axon/libaxon_pjrt.so")
            if hook is not None:
                ah.set_axon_ntff_profile_hook(hook)
        except Exception:
            pass
    import concourse.bass_utils as bu

    bu.upload_artifacts = lambda tmpdir: f"local:{tmpdir}"


def kernel(prob_class, predict_bbox, labels):
    global LAST_EXEC_NS, LAST_RESULTS
    if "nc" not in _CACHE:
        _CACHE["nc"] = _build_program()
    nc = _CACHE["nc"]

    in_maps = _host_prep(prob_class, predict_bbox, labels)
    if TRACE:
        _install_profile_shim()
    res = run_bass_kernel_spmd(nc, in_maps, list(range(NCORES)), trace=TRACE)
    LAST_EXEC_NS = res.exec_time_ns
    LAST_RESULTS = res

    cost = np.empty((B, P, T), np.float32)
    for core in range(NCORES):
        cst = res.results[core]["cost"]    # [128, NTILES, T]
        nll = res.results[core]["nllm"]    # [128, NSEG, T]
        slab = np.empty((ROWS, T), np.float32)
        for k in range(NTILES):
            r0, r1 = k * 128, min(k * 128 + 128, ROWS)
            slab[r0:r1] = cst[0:r1 - r0, k, :]
        for sidx, (k, si, b, p0, ln, off) in enumerate(SEGS):
            r0 = k * 128 + off
            slab[r0:r0 + ln] += nll[0:ln, sidx, :]
        cost[core * BL:(core + 1) * BL] = slab.reshape(BL, P, T)

    loss = np.float32(0.0)
    cost64 = cost.astype(np.float64)
    for i in range(B):
        r, c = _lsa(cost64[i])
        loss = loss + cost[:, r, c].mean(dtype=np.float32)
    return np.float32(loss)


# revision 18
# speedup vs baseline: 1.0075x; 1.0075x over previous
"""DETR-style matching loss on 8 Trainium2 NeuronCores.

Device (data-parallel over batch, 8 samples/core): the [B,P,T] pairwise
cost matrix  cost = NLL + mask*(5*L1 + 2*GIoU_loss)  computed per core as
a [1200, 150] slab (rows = (b,p) packed, 10 tiles of 128 partitions).
Host: scipy Hungarian per sample on the device cost + the final
gather/mean (inherently sequential scalar work on 150x150 matrices).

v4 design:
- All per-(row,t) bilinear terms come from one bf16 3-pass PE matmul per
  PSUM bank (exact to ~2^-25): label-quantity broadcasts AND pairwise
  pre-sums (areal+apE, lwE+pwE, lhE+phE) AND pairwise pre-diffs for L1.
  Per bank: K=15 (6 selector rows = 2 segments x hi/mid/lo + up to 3
  pred-side values x hi/mid/lo against slot-indicator rows), N=450.
- NLL + 4*mask from an fp32 one-hot matmul (exact: one product/output).
- The nonlinear chain runs as fused custom-DVE ops + stock tensor ops
  balanced across DVE/GpSimd/ScalarE.
- DMA is batched: 3 whole-kernel input DMAs (pt/rhs/scal, b or tile
  packed into a middle axis), 1 bc DMA per tile, and cost/nll
  accumulated in SBUF slabs shipped out in 3 chunked DMAs each.

Math (per pair, r=(b,p) row, t):
  iw = min(px1,lx1) - max(px0,lx0);  ih likewise;  inter = relu.iw*relu.ih
  ue = (areal + area_p + EPS) - inter          [= union + EPS]
  ew = (lwE + pwE) - iw;  eh = (lhE + phE) - ih   [min+max=sum identity]
  ee = ew*eh + EPS                              [= enclose + EPS]
  rc = 1/(ue*ee)   [approx recip + 1 Newton step]
  cost = [nll + 4*mask] + 2*mask*( 0.625*sum|d| - (inter*ee + ue^2)*rc )
"""

import sys

import numpy as np

for _p in ("/opt/trn_rl_repo",):
    if _p not in sys.path:
        sys.path.append(_p)

import ml_dtypes

import concourse.bacc as bacc
import concourse.tile as tile
from concourse import mybir
from concourse.bass_utils import run_bass_kernel_spmd

F32 = mybir.dt.float32
BF16 = mybir.dt.bfloat16
NPBF = ml_dtypes.bfloat16
EPS = np.float32(1e-7)
IMG_SIZE = np.float32(320.0)
NO_OBJ = 80

B, P, T, C = 64, 150, 150, 81
NCORES = 8
BL = B // NCORES          # 8 samples per core
ROWS = BL * P             # 1200 (b,p) rows per core
NTILES = (ROWS + 127) // 128   # 10
NK = 15                   # bc-matmul contraction rows per bank
KN = 96                   # nll contraction rows (81 lnp + 15 ones-pad)

# bc slot layout: slot q of bank j=q//3 at psum col 512*j + 150*(q%3).
# bank0 stays PSUM-resident; banks 1-3 are copied to SBUF.
SLOT = {"lx1": 0, "ly1": 1, "dcx": 2,
        "lx0": 3, "ly0": 4, "dcy": 5,
        "dw": 6, "dh": 7, "mask2": 8,
        "uepre": 9, "ewpre": 10, "ehpre": 11}
# slot -> pred-side presum value name (None = no pred part)
PRESUM_OF = {"dcx": "npcx", "dcy": "npcy", "dw": "npw", "dh": "nph",
             "uepre": "apE", "ewpre": "pwE", "ehpre": "phE"}


def _tile_segments():
    tiles = []
    for k in range(NTILES):
        r0, r1 = k * 128, min(k * 128 + 128, ROWS)
        segs, r = [], r0
        while r < r1:
            b = r // P
            p0 = r % P
            ln = min(r1 - r, P - p0)
            segs.append((b, p0, ln, r - r0))
            r += ln
        tiles.append(segs)
    return tiles


TILES = _tile_segments()
# flat segment list for the nll slab pages
SEGS = [(k, si, b, p0, ln, off)
        for k, segs in enumerate(TILES)
        for si, (b, p0, ln, off) in enumerate(segs)]
NSEG = len(SEGS)   # 17

_CACHE = {}
TRACE = False
LAST_EXEC_NS = None
LAST_RESULTS = None


def _register_custom_ops():
    if "ops" in _CACHE:
        return _CACHE["ops"]
    from concourse import dve_ops
    from concourse.dve_spec import (
        Spec, Src0, Src1, C0, C1, C2, Zero, relu, sq, maxx, minn, lower,
        _has_src1,
    )
    from concourse.dve_uop import DveOpSpec

    def ref_iw(in0, in1, s0, s1, imm2):
        return np.minimum(in0, s0) - np.maximum(in1, s1)

    def ref_relumul(in0, in1, s0, s1, imm2):
        return np.maximum(in0, 0) * np.maximum(in1, 0)

    def ref_fmaeps(in0, in1, s0, s1, imm2):
        return in0 * in1 + imm2

    def ref_addsq(in0, in1, s0, s1, imm2):
        return in0 + in1 * in1

    def ref_abs2(in0, in1, s0, s1, imm2):
        return (np.abs(in0) + np.abs(in1)) * imm2

    specs = {
        "DETR_IW": Spec(body=minn(Src0, C0) - maxx(Src1, C1),
                        reference=ref_iw),
        "DETR_RELUMUL": Spec(body=relu(Src0) * relu(Src1),
                             reference=ref_relumul),
        "DETR_FMAEPS": Spec(body=Src0 * Src1 + C2, reference=ref_fmaeps),
        "DETR_ADDSQ": Spec(body=Src0 + sq(Src1), reference=ref_addsq),
        "DETR_ABS2": Spec(
            body=(maxx(Src0, Zero - Src0) + maxx(Src1, Zero - Src1)) * C2,
            reference=ref_abs2),
    }
    ops = {}
    existing = {op.name for op in dve_ops.OPS}
    for name, spec in specs.items():
        if name in existing:
            ops[name] = next(o for o in dve_ops.OPS if o.name == name)
            continue
        shas = {}
        for ver in ("v3", "v4"):
            try:
                uops = lower(spec, ver=ver)
                s = DveOpSpec(name=name, opcode=1, uops=uops,
                              rd1_en=_has_src1(spec))
                shas[ver] = s.sha(ver)
            except Exception:
                pass
        op = dve_ops.DveOp(name, spec, subdim=False, uops_sha=shas)
        dve_ops.OPS.append(op)
        dve_ops.CUSTOM_DVE_SPECS[name] = spec
        dve_ops._SUB_OPCODE_FOR_NAME[name] = (
            max(dve_ops._SUB_OPCODE_FOR_NAME.values()) + 1)
        ops[name] = op
    _CACHE["ops"] = ops
    return ops


def _build_program():
    ops = _register_custom_ops()
    nc = bacc.Bacc(None, target_bir_lowering=False)

    # batched inputs: b (or tile) packed into a middle axis for few, fat DMAs
    pt_d = nc.declare_dram_parameter("pt", [C, BL, P], F32, isOutput=False)
    rhs_d = nc.declare_dram_parameter("rhsnm", [KN, BL, T], F32,
                                      isOutput=False)
    scal_d = nc.declare_dram_parameter("scal", [128, NTILES, 4], F32,
                                       isOutput=False)
    bcr_d = nc.declare_dram_parameter("bcr", [NTILES, NK, 2312], BF16,
                                      isOutput=False)
    cost_d = nc.declare_dram_parameter("cost", [128, NTILES, T], F32,
                                       isOutput=True)
    nll_d = nc.declare_dram_parameter("nllm", [128, NSEG, T], F32,
                                      isOutput=True)

    from contextlib import ExitStack

    with tile.TileContext(nc) as tc, ExitStack() as ctx:
        consts = ctx.enter_context(tc.tile_pool(name="consts", bufs=1))
        perb = ctx.enter_context(tc.tile_pool(name="perb", bufs=3))
        pert = ctx.enter_context(tc.tile_pool(name="pert", bufs=3))
        bcs = ctx.enter_context(tc.tile_pool(name="bcs", bufs=3))
        work = ctx.enter_context(tc.tile_pool(name="work", bufs=3))
        slab = ctx.enter_context(tc.tile_pool(name="slab", bufs=1))
        bcp0 = ctx.enter_context(tc.tile_pool(name="bcp0", bufs=2,
                                              space="PSUM"))
        bcp = ctx.enter_context(tc.tile_pool(name="bcp", bufs=1, space="PSUM"))
        nll0p = ctx.enter_context(tc.tile_pool(name="nll0p", bufs=2,
                                               space="PSUM"))
        nll1p = ctx.enter_context(tc.tile_pool(name="nll1p", bufs=1,
                                               space="PSUM"))

        v = nc.vector
        g = nc.gpsimd
        s = nc.scalar

        # whole-kernel batched inputs
        ptall = consts.tile([C, BL, P], F32)
        nc.sync.dma_start(out=ptall[:], in_=pt_d[:])
        rsall = consts.tile([KN, BL, T], F32)
        nc.sync.dma_start(out=rsall[:], in_=rhs_d[:])
        scall = consts.tile([128, NTILES, 4], F32)
        nc.sync.dma_start(out=scall[:], in_=scal_d[:])

        # output slabs
        costall = slab.tile([128, NTILES, T], F32)
        nllall = slab.tile([128, NSEG, T], F32)

        lhs = {}

        def ensure_b(b):
            if b in lhs:
                return
            lt = perb.tile([KN, P], F32, tag="lhs")
            nc.vector.memset(lt[64:KN, :], 1.0)
            nc.scalar.activation(out=lt[0:C, :], in_=ptall[:, b, :],
                                 func=mybir.ActivationFunctionType.Ln)
            lhs[b] = lt

        segidx = 0
        for k, segs in enumerate(TILES):
            nrow = sum(sg[2] for sg in segs)
            for b, _, _, _ in segs:
                ensure_b(b)

            # ---- bc matmuls: one per PSUM bank, K=15 bf16, N=450 ----
            bt = pert.tile([NK, 2312], BF16, tag="bt")
            nc.sync.dma_start(out=bt[:], in_=bcr_d[k])
            ps0 = bcp0.tile([128, 512], F32, tag="bcps0")
            ps = bcp.tile([128, 1536], F32, tag="bcps")
            for j in range(4):
                out = (ps0[:, 0:450] if j == 0
                       else ps[:, 512 * (j - 1):512 * (j - 1) + 450])
                nc.tensor.matmul(out,
                                 bt[:, 1800 + 128 * j:1800 + 128 * (j + 1)],
                                 bt[:, 450 * j:450 * (j + 1)],
                                 start=True, stop=True)

            # copy banks 1-3 to SBUF (split DVE / ACT) — frees ps for k+1
            sb = bcs.tile([128, 1536], F32, tag="bcsb")
            v.tensor_copy(sb[:, 0:512], ps[:, 0:512])
            s.copy(sb[:, 512:1536], ps[:, 512:1536])

            def bcP(name):
                off = 150 * (SLOT[name] % 3)
                return ps0[0:nrow, off:off + T]

            def bcS(name):
                q = SLOT[name]
                off = 512 * (q // 3 - 1) + 150 * (q % 3)
                return sb[0:nrow, off:off + T]

            # ---- nll + 4*mask (fp32 one-hot matmul per segment) ----
            for si, (b, p0, ln, off) in enumerate(segs):
                pool_ = nll0p if si == 0 else nll1p
                nm = pool_.tile([128, 512], F32, tag=f"nll{si}",
                                name=f"nll{si}")
                nc.tensor.matmul(nm[0:ln, 0:T], lhs[b][:, p0:p0 + ln],
                                 rsall[:, b, :], start=True, stop=True)
                s.copy(out=nllall[0:ln, segidx, :], in_=nm[0:ln, 0:T])
                segidx += 1

            # ---- pred xyxy scalars ----
            px1, px0, py1, py0 = (scall[0:nrow, k, i:i + 1] for i in range(4))

            def wt(tag, w=T):
                return work.tile([128, w], F32, tag=tag, name=tag)

            # ---- fused chain ----
            t3 = wt("t3", 3 * T)
            iw = t3[0:nrow, T:2 * T]
            ih = t3[0:nrow, 2 * T:3 * T]
            inter = t3[0:nrow, 0:T]
            v._custom_dve(ops["DETR_IW"], out=iw, in0=bcP("lx1"),
                          in1=bcS("lx0"), s0=px1, s1=px0)
            v._custom_dve(ops["DETR_IW"], out=ih, in0=bcP("ly1"),
                          in1=bcS("ly0"), s0=py1, s1=py0)
            v._custom_dve(ops["DETR_RELUMUL"], out=inter, in0=iw, in1=ih)

            # [ue|ew|eh] = [uepre|ewpre|ehpre] - [inter|iw|ih]  (one GPS op)
            uew = wt("uew", 3 * T)
            g.tensor_sub(uew[0:nrow, :], sb[0:nrow, 1024:1024 + 3 * T],
                         t3[0:nrow, :])
            ue = uew[0:nrow, 0:T]
            ew = uew[0:nrow, T:2 * T]
            eh = uew[0:nrow, 2 * T:3 * T]

            ee = wt("ee")[0:nrow, :]
            v._custom_dve(ops["DETR_FMAEPS"], out=ee, in0=ew, in1=eh,
                          imm2=float(EPS))
            prod = wt("prod")[0:nrow, :]
            v.tensor_mul(prod, ue, ee)
            t1 = wt("t1")[0:nrow, :]
            v.tensor_mul(t1, inter, ee)
            r0 = wt("r0")[0:nrow, :]
            rc = wt("rc")[0:nrow, :]
            v.reciprocal_approx_accurate(out=rc, in_=prod, scratch=r0)
            gnum = wt("gnum")[0:nrow, :]
            v._custom_dve(ops["DETR_ADDSQ"], out=gnum, in0=t1, in1=ue)
            gterm = wt("gterm")[0:nrow, :]
            g.tensor_mul(gterm, gnum, rc)

            s01 = wt("s01")[0:nrow, :]
            v._custom_dve(ops["DETR_ABS2"], out=s01, in0=bcP("dcx"),
                          in1=bcS("dcy"), imm2=0.625)
            s23 = wt("s23")[0:nrow, :]
            v._custom_dve(ops["DETR_ABS2"], out=s23, in0=bcS("dw"),
                          in1=bcS("dh"), imm2=0.625)
            l1t = wt("l1t")[0:nrow, :]
            g.tensor_add(l1t, s01, s23)
            pre = wt("pre")[0:nrow, :]
            g.tensor_sub(pre, l1t, gterm)
            g.tensor_mul(costall[0:nrow, k, :], pre, bcS("mask2"))

            # chunked slab flushes
            if k == NTILES - 1:
                for lo, hi in ((0, 4), (4, 7), (7, 10)):
                    nc.sync.dma_start(out=cost_d[:, lo:hi, :],
                                      in_=costall[:, lo:hi, :])
                for lo, hi in ((0, 7), (7, 12), (12, NSEG)):
                    nc.sync.dma_start(out=nll_d[:, lo:hi, :],
                                      in_=nllall[:, lo:hi, :])

    nc.finalize()
    return nc


def _decomp3(vv):
    """f32 -> three bf16 arrays summing to v (error ~2^-25 |v|)."""
    vv = np.asarray(vv, np.float32)
    h = vv.astype(NPBF)
    r = vv - h.astype(np.float32)
    m = r.astype(NPBF)
    lo = (r - m.astype(np.float32)).astype(NPBF)
    return h, m, lo


def _host_prep(prob_class, predict_bbox, labels):
    pc = np.asarray(prob_class, np.float32)
    pb = np.asarray(predict_bbox, np.float32)
    lab = np.asarray(labels, np.float32)

    lb = lab[..., :4] / IMG_SIZE
    cls = lab[..., 4].astype(np.int32)
    mask = (cls != NO_OBJ).astype(np.float32)

    lcx, lcy, lw, lh = (lb[..., i] for i in range(4))
    half = np.float32(0.5)
    lx0 = lcx - half * lw
    lx1 = lcx + half * lw
    ly0 = lcy - half * lh
    ly1 = lcy + half * lh
    lwE = lx1 - lx0
    lhE = ly1 - ly0
    areal = lwE * lhE

    labvals = {"lx1": lx1, "ly1": ly1, "dcx": lcx,
               "lx0": lx0, "ly0": ly0, "dcy": lcy,
               "dw": lw, "dh": lh, "mask2": 2.0 * mask,
               "uepre": areal, "ewpre": lwE, "ehpre": lhE}
    qlab = np.zeros((B, 1800), np.float32)
    for name, q in SLOT.items():
        off = 450 * (q // 3) + 150 * (q % 3)
        qlab[:, off:off + T] = labvals[name]

    pcx, pcy, pw, ph = (pb[..., i] for i in range(4))
    px0 = pcx - half * pw
    px1 = pcx + half * pw
    py0 = pcy - half * ph
    py1 = pcy + half * ph
    pwE = px1 - px0
    phE = py1 - py0
    apE = pwE * phE + EPS
    presvals = {"npcx": -pcx, "npcy": -pcy, "npw": -pw, "nph": -ph,
                "apE": apE, "pwE": pwE, "phE": phE}   # [B, P] each

    # scal packed [128, NTILES, 4] per core (row r of tile k = global row
    # 128k + r)
    scal_rows = np.stack([px1, px0, py1, py0], axis=-1).reshape(
        NCORES, ROWS, 4)
    scal = np.zeros((NCORES, 128, NTILES, 4), np.float32)
    for k in range(NTILES):
        r0, r1 = k * 128, min(k * 128 + 128, ROWS)
        scal[:, 0:r1 - r0, k, :] = scal_rows[:, r0:r1, :]

    # nll rhs [KN, B-local, T] per core
    rhsnm = np.zeros((B, KN, T), np.float32)
    bi = np.repeat(np.arange(B), T)
    ti = np.tile(np.arange(T), B)
    rhsnm[bi, cls.ravel(), ti] = np.float32(-1.0)
    rhsnm[:, 81, :] = np.float32(4.0) * mask
    rhs_t = np.ascontiguousarray(
        rhsnm.reshape(NCORES, BL, KN, T).transpose(0, 2, 1, 3))

    # ln arg [C, B-local, P] per core
    pt_t = np.ascontiguousarray(
        pc.reshape(NCORES, BL, P, C).transpose(0, 3, 1, 2))

    qlab3 = _decomp3(qlab)                       # 3 x [B, 1800]
    pres3 = {n: _decomp3(val) for n, val in presvals.items()}

    bcr = np.zeros((NCORES, NTILES, NK, 2312), NPBF)
    for k, segs in enumerate(TILES):
        for core in range(NCORES):
            for si, (b, p0, ln, off) in enumerate(segs):
                gb = core * BL + b
                for p3 in range(3):
                    # label rows: values on rhs cols, sel on lhsT cols
                    bcr[core, k, 3 * si + p3, 0:1800] = qlab3[p3][gb]
                    for j in range(4):
                        bcr[core, k, 3 * si + p3,
                            1800 + 128 * j + off:1800 + 128 * j + off + ln] \
                            = NPBF(1.0)
            # presum rows 6..14: per-bank slot m
            for name, q in SLOT.items():
                pn = PRESUM_OF.get(name)
                if pn is None:
                    continue
                j, m = q // 3, q % 3
                coff = 450 * j + 150 * m
                for si, (b, p0, ln, off) in enumerate(segs):
                    gb = core * BL + b
                    for p3 in range(3):
                        row = 6 + 3 * m + p3
                        bcr[core, k, row,
                            1800 + 128 * j + off:1800 + 128 * j + off + ln] \
                            = pres3[pn][p3][gb, p0:p0 + ln]
                        bcr[core, k, row, coff:coff + T] = NPBF(1.0)

    in_maps = []
    for core in range(NCORES):
        in_maps.append({
            "pt": pt_t[core],
            "rhsnm": rhs_t[core],
            "bcr": bcr[core],
            "scal": scal[core],
        })
    return in_maps


def _hungarian_np(cost):
    """Jonker-Volgenant LSA fallback (same algorithm as scipy)."""
    cost = np.asarray(cost, dtype=np.float64)
    n, m = cost.shape
    INF = float("inf")
    u = np.zeros(n + 1)
    vv = np.zeros(m + 1)
    p = np.zeros(m + 1, dtype=np.int64)
    way = np.zeros(m + 1, dtype=np.int64)
    for i in range(1, n + 1):
        p[0] = i
        j0 = 0
        minv = np.full(m + 1, INF)
        used = np.zeros(m + 1, dtype=bool)
        while True:
            used[j0] = True
            i0 = p[j0]
            free = ~used[1:]
            cur = cost[i0 - 1] - u[i0] - vv[1:]
            better = free & (cur < minv[1:])
            minv[1:] = np.where(better, cur, minv[1:])
            way[1:] = np.where(better, j0, way[1:])
            masked = np.where(free, minv[1:], INF)
            j1 = int(np.argmin(masked)) + 1
            delta = masked[j1 - 1]
            uj = np.nonzero(used)[0]
            u[p[uj]] += delta
            vv[uj] -= delta
            minv[1:] = np.where(free, minv[1:] - delta, minv[1:])
            j0 = j1
            if p[j0] == 0:
                break
        while j0 != 0:
            j1 = way[j0]
            p[j0] = p[j1]
            j0 = j1
    cols = np.nonzero(p[1:])[0]
    rows = p[1:][cols] - 1
    order = np.argsort(rows)
    return rows[order], cols[order]


def _lsa(cost):
    try:
        from scipy.optimize import linear_sum_assignment
        return linear_sum_assignment(cost)
    except Exception:
        return _hungarian_np(cost)


def _install_profile_shim():
    import types

    if "antenv.axon_hooks" not in sys.modules:
        import antenv

        mod = types.ModuleType("antenv.axon_hooks")
        mod._HOOK = None
        mod.set_axon_ntff_profile_hook = lambda h: setattr(mod, "_HOOK", h)
        mod.get_axon_ntff_profile_hook = lambda: mod._HOOK
        sys.modules["antenv.axon_hooks"] = mod
        antenv.axon_hooks = mod
    import antenv.axon_hooks as ah

    if ah.get_axon_ntff_profile_hook() is None:
        try:
            from trn_agent_boot.trn_boot import _ntff_profile_via_ctypes

            hook = _ntff_profile_via_ctypes("/opt/axon/libaxon_pjrt.so")
            if hook is not None:
                ah.set_axon_ntff_profile_hook(hook)
        except Exception:
            pass
    import concourse.bass_utils as bu

    bu.upload_artifacts = lambda tmpdir: f"local:{tmpdir}"


def kernel(prob_class, predict_bbox, labels):
    global LAST_EXEC_NS, LAST_RESULTS
    if "nc" not in _CACHE:
        _CACHE["nc"] = _build_program()
    nc = _CACHE["nc"]

    in_maps = _host_prep(prob_class, predict_bbox, labels)
    if TRACE:
        _install_profile_shim()
    res = run_bass_kernel_spmd(nc, in_maps, list(range(NCORES)), trace=TRACE)
    LAST_EXEC_NS = res.exec_time_ns
    LAST_RESULTS = res

    cost = np.empty((B, P, T), np.float32)
    for core in range(NCORES):
        cst = res.results[core]["cost"]    # [128, NTILES, T]
        nll = res.results[core]["nllm"]    # [128, NSEG, T]
        slab = np.empty((ROWS, T), np.float32)
        for k in range(NTILES):
            r0, r1 = k * 128, min(k * 128 + 128, ROWS)
            slab[r0:r1] = cst[0:r1 - r0, k, :]
        for sidx, (k, si, b, p0, ln, off) in enumerate(SEGS):
            r0 = k * 128 + off
            slab[r0:r0 + ln] += nll[0:ln, sidx, :]
        cost[core * BL:(core + 1) * BL] = slab.reshape(BL, P, T)

    loss = np.float32(0.0)
    cost64 = cost.astype(np.float64)
    for i in range(B):
        r, c = _lsa(cost64[i])
        loss = loss + cost[:, r, c].mean(dtype=np.float32)
    return np.float32(loss)


# revision 19
# speedup vs baseline: 1.1105x; 1.1022x over previous
"""DETR-style matching loss on 8 Trainium2 NeuronCores.

Device (data-parallel over batch, 8 samples/core): the [B,P,T] pairwise
cost matrix  cost = NLL + mask*(5*L1 + 2*GIoU_loss)  computed per core as
a [1200, 150] slab (rows = (b,p) packed, 10 tiles of 128 partitions).
Host: scipy Hungarian per sample on the device cost + the final
gather/mean (inherently sequential scalar work on 150x150 matrices).

v4 design:
- All per-(row,t) bilinear terms come from one bf16 3-pass PE matmul per
  PSUM bank (exact to ~2^-25): label-quantity broadcasts AND pairwise
  pre-sums (areal+apE, lwE+pwE, lhE+phE) AND pairwise pre-diffs for L1.
  Per bank: K=15 (6 selector rows = 2 segments x hi/mid/lo + up to 3
  pred-side values x hi/mid/lo against slot-indicator rows), N=450.
- NLL + 4*mask from an fp32 one-hot matmul (exact: one product/output).
- The nonlinear chain runs as fused custom-DVE ops + stock tensor ops
  balanced across DVE/GpSimd/ScalarE.
- DMA is batched: 3 whole-kernel input DMAs (pt/rhs/scal, b or tile
  packed into a middle axis), 1 bc DMA per tile, and cost/nll
  accumulated in SBUF slabs shipped out in 3 chunked DMAs each.

Math (per pair, r=(b,p) row, t):
  iw = min(px1,lx1) - max(px0,lx0);  ih likewise;  inter = relu.iw*relu.ih
  ue = (areal + area_p + EPS) - inter          [= union + EPS]
  ew = (lwE + pwE) - iw;  eh = (lhE + phE) - ih   [min+max=sum identity]
  ee = ew*eh + EPS                              [= enclose + EPS]
  rc = 1/(ue*ee)   [approx recip + 1 Newton step]
  cost = [nll + 4*mask] + 2*mask*( 0.625*sum|d| - (inter*ee + ue^2)*rc )
"""

import sys

import numpy as np

for _p in ("/opt/trn_rl_repo",):
    if _p not in sys.path:
        sys.path.append(_p)

import ml_dtypes

import concourse.bacc as bacc
import concourse.tile as tile
from concourse import mybir
from concourse.bass_utils import run_bass_kernel_spmd

F32 = mybir.dt.float32
BF16 = mybir.dt.bfloat16
NPBF = ml_dtypes.bfloat16
EPS = np.float32(1e-7)
IMG_SIZE = np.float32(320.0)
NO_OBJ = 80

B, P, T, C = 64, 150, 150, 81
NCORES = 8
BL = B // NCORES          # 8 samples per core
ROWS = BL * P             # 1200 (b,p) rows per core
NTILES = (ROWS + 127) // 128   # 10
NK = 15                   # bc-matmul contraction rows per bank
KN = 96                   # nll contraction rows (81 lnp + 15 ones-pad)

# bc slot layout: slot q of bank j=q//3 at psum col 512*j + 150*(q%3).
# bank0 stays PSUM-resident; banks 1-3 are copied to SBUF.
SLOT = {"lx1": 0, "ly1": 1, "dcx": 2,
        "lx0": 3, "ly0": 4, "dcy": 5,
        "dw": 6, "dh": 7, "mask2": 8,
        "uepre": 9, "ewpre": 10, "ehpre": 11}
# slot -> pred-side presum value name (None = no pred part)
PRESUM_OF = {"dcx": "npcx", "dcy": "npcy", "dw": "npw", "dh": "nph",
             "uepre": "apE", "ewpre": "pwE", "ehpre": "phE"}


def _tile_segments():
    tiles = []
    for k in range(NTILES):
        r0, r1 = k * 128, min(k * 128 + 128, ROWS)
        segs, r = [], r0
        while r < r1:
            b = r // P
            p0 = r % P
            ln = min(r1 - r, P - p0)
            segs.append((b, p0, ln, r - r0))
            r += ln
        tiles.append(segs)
    return tiles


TILES = _tile_segments()
# flat segment list for the nll slab pages
SEGS = [(k, si, b, p0, ln, off)
        for k, segs in enumerate(TILES)
        for si, (b, p0, ln, off) in enumerate(segs)]
NSEG = len(SEGS)   # 17

_CACHE = {}
TRACE = False
LAST_EXEC_NS = None
LAST_RESULTS = None


def _register_custom_ops():
    if "ops" in _CACHE:
        return _CACHE["ops"]
    from concourse import dve_ops
    from concourse.dve_spec import (
        Spec, Src0, Src1, C0, C1, C2, Zero, relu, sq, maxx, minn, lower,
        _has_src1,
    )
    from concourse.dve_uop import DveOpSpec

    def ref_iw(in0, in1, s0, s1, imm2):
        return np.minimum(in0, s0) - np.maximum(in1, s1)

    def ref_relumul(in0, in1, s0, s1, imm2):
        return np.maximum(in0, 0) * np.maximum(in1, 0)

    def ref_fmaeps(in0, in1, s0, s1, imm2):
        return in0 * in1 + imm2

    def ref_addsq(in0, in1, s0, s1, imm2):
        return in0 + in1 * in1

    def ref_abs2(in0, in1, s0, s1, imm2):
        return (np.abs(in0) + np.abs(in1)) * imm2

    specs = {
        "DETR_IW": Spec(body=minn(Src0, C0) - maxx(Src1, C1),
                        reference=ref_iw),
        "DETR_RELUMUL": Spec(body=relu(Src0) * relu(Src1),
                             reference=ref_relumul),
        "DETR_FMAEPS": Spec(body=Src0 * Src1 + C2, reference=ref_fmaeps),
        "DETR_ADDSQ": Spec(body=Src0 + sq(Src1), reference=ref_addsq),
        "DETR_ABS2": Spec(
            body=(maxx(Src0, Zero - Src0) + maxx(Src1, Zero - Src1)) * C2,
            reference=ref_abs2),
    }
    ops = {}
    existing = {op.name for op in dve_ops.OPS}
    for name, spec in specs.items():
        if name in existing:
            ops[name] = next(o for o in dve_ops.OPS if o.name == name)
            continue
        shas = {}
        for ver in ("v3", "v4"):
            try:
                uops = lower(spec, ver=ver)
                s = DveOpSpec(name=name, opcode=1, uops=uops,
                              rd1_en=_has_src1(spec))
                shas[ver] = s.sha(ver)
            except Exception:
                pass
        op = dve_ops.DveOp(name, spec, subdim=False, uops_sha=shas)
        dve_ops.OPS.append(op)
        dve_ops.CUSTOM_DVE_SPECS[name] = spec
        dve_ops._SUB_OPCODE_FOR_NAME[name] = (
            max(dve_ops._SUB_OPCODE_FOR_NAME.values()) + 1)
        ops[name] = op
    _CACHE["ops"] = ops
    return ops


def _build_program():
    ops = _register_custom_ops()
    nc = bacc.Bacc(None, target_bir_lowering=False)

    # batched inputs: b (or tile) packed into a middle axis for few, fat DMAs
    pt_d = nc.declare_dram_parameter("pt", [C, BL, P], F32, isOutput=False)
    rhs_d = nc.declare_dram_parameter("rhsnm", [KN, BL, T], F32,
                                      isOutput=False)
    scal_d = nc.declare_dram_parameter("scal", [128, NTILES, 4], F32,
                                       isOutput=False)
    bcr_d = nc.declare_dram_parameter("bcr", [NTILES, NK, 2312], BF16,
                                      isOutput=False)
    cost_d = nc.declare_dram_parameter("cost", [128, NTILES, T], F32,
                                       isOutput=True)
    nll_d = nc.declare_dram_parameter("nllm", [128, NSEG, T], F32,
                                      isOutput=True)

    from contextlib import ExitStack

    with tile.TileContext(nc) as tc, ExitStack() as ctx:
        consts = ctx.enter_context(tc.tile_pool(name="consts", bufs=1))
        perb = ctx.enter_context(tc.tile_pool(name="perb", bufs=3))
        pert = ctx.enter_context(tc.tile_pool(name="pert", bufs=3))
        bcs = ctx.enter_context(tc.tile_pool(name="bcs", bufs=4))
        work = ctx.enter_context(tc.tile_pool(name="work", bufs=4))
        slab = ctx.enter_context(tc.tile_pool(name="slab", bufs=1))
        bcp0 = ctx.enter_context(tc.tile_pool(name="bcp0", bufs=2,
                                              space="PSUM"))
        bcp = ctx.enter_context(tc.tile_pool(name="bcp", bufs=1, space="PSUM"))
        nll0p = ctx.enter_context(tc.tile_pool(name="nll0p", bufs=2,
                                               space="PSUM"))
        nll1p = ctx.enter_context(tc.tile_pool(name="nll1p", bufs=1,
                                               space="PSUM"))

        v = nc.vector
        g = nc.gpsimd
        s = nc.scalar

        # whole-kernel batched inputs
        ptall = consts.tile([C, BL, P], F32)
        nc.sync.dma_start(out=ptall[:], in_=pt_d[:])
        rsall = consts.tile([KN, BL, T], F32)
        nc.sync.dma_start(out=rsall[:], in_=rhs_d[:])
        scall = consts.tile([128, NTILES, 4], F32)
        nc.sync.dma_start(out=scall[:], in_=scal_d[:])

        # output slabs
        costall = slab.tile([128, NTILES, T], F32)
        nllall = slab.tile([128, NSEG, T], F32)

        lhs = {}

        def ensure_b(b):
            if b in lhs:
                return
            lt = perb.tile([KN, P], F32, tag="lhs")
            nc.vector.memset(lt[64:KN, :], 1.0)
            nc.scalar.activation(out=lt[0:C, :], in_=ptall[:, b, :],
                                 func=mybir.ActivationFunctionType.Ln)
            lhs[b] = lt

        segidx = 0
        for k, segs in enumerate(TILES):
            nrow = sum(sg[2] for sg in segs)
            for b, _, _, _ in segs:
                ensure_b(b)

            # ---- bc matmuls: one per PSUM bank, K=15 bf16, N=450 ----
            bt = pert.tile([NK, 2312], BF16, tag="bt")
            nc.sync.dma_start(out=bt[:], in_=bcr_d[k])
            ps0 = bcp0.tile([128, 512], F32, tag="bcps0")
            ps = bcp.tile([128, 1536], F32, tag="bcps")
            for j in range(4):
                out = (ps0[:, 0:450] if j == 0
                       else ps[:, 512 * (j - 1):512 * (j - 1) + 450])
                nc.tensor.matmul(out,
                                 bt[:, 1800 + 128 * j:1800 + 128 * (j + 1)],
                                 bt[:, 450 * j:450 * (j + 1)],
                                 start=True, stop=True)

            # copy banks 1-3 to SBUF (split DVE / ACT) — frees ps for k+1
            sb = bcs.tile([128, 1536], F32, tag="bcsb")
            s.copy(sb[:, 0:512], ps[:, 0:512])
            s.copy(sb[:, 512:1536], ps[:, 512:1536])

            def bcP(name):
                off = 150 * (SLOT[name] % 3)
                return ps0[0:nrow, off:off + T]

            def bcS(name):
                q = SLOT[name]
                off = 512 * (q // 3 - 1) + 150 * (q % 3)
                return sb[0:nrow, off:off + T]

            # ---- nll + 4*mask (fp32 one-hot matmul per segment) ----
            for si, (b, p0, ln, off) in enumerate(segs):
                pool_ = nll0p if si == 0 else nll1p
                nm = pool_.tile([128, 512], F32, tag=f"nll{si}",
                                name=f"nll{si}")
                nc.tensor.matmul(nm[0:ln, 0:T], lhs[b][:, p0:p0 + ln],
                                 rsall[:, b, :], start=True, stop=True)
                s.copy(out=nllall[0:ln, segidx, :], in_=nm[0:ln, 0:T])
                segidx += 1

            # ---- pred xyxy scalars ----
            px1, px0, py1, py0 = (scall[0:nrow, k, i:i + 1] for i in range(4))

            def wt(tag, w=T):
                return work.tile([128, w], F32, tag=tag, name=tag)

            # ---- fused chain ----
            t3 = wt("t3", 3 * T)
            iw = t3[0:nrow, T:2 * T]
            ih = t3[0:nrow, 2 * T:3 * T]
            inter = t3[0:nrow, 0:T]
            v._custom_dve(ops["DETR_IW"], out=iw, in0=bcP("lx1"),
                          in1=bcS("lx0"), s0=px1, s1=px0)
            v._custom_dve(ops["DETR_IW"], out=ih, in0=bcP("ly1"),
                          in1=bcS("ly0"), s0=py1, s1=py0)
            v._custom_dve(ops["DETR_RELUMUL"], out=inter, in0=iw, in1=ih)

            s01 = wt("s01")[0:nrow, :]
            v._custom_dve(ops["DETR_ABS2"], out=s01, in0=bcP("dcx"),
                          in1=bcS("dcy"), imm2=0.625)
            s23 = wt("s23")[0:nrow, :]
            v._custom_dve(ops["DETR_ABS2"], out=s23, in0=bcS("dw"),
                          in1=bcS("dh"), imm2=0.625)

            # [ue|ew|eh] = [uepre|ewpre|ehpre] - [inter|iw|ih]  (one GPS op)
            uew = wt("uew", 3 * T)
            g.tensor_sub(uew[0:nrow, :], sb[0:nrow, 1024:1024 + 3 * T],
                         t3[0:nrow, :])
            ue = uew[0:nrow, 0:T]
            ew = uew[0:nrow, T:2 * T]
            eh = uew[0:nrow, 2 * T:3 * T]

            ee = wt("ee")[0:nrow, :]
            v._custom_dve(ops["DETR_FMAEPS"], out=ee, in0=ew, in1=eh,
                          imm2=float(EPS))
            prod = wt("prod")[0:nrow, :]
            v.tensor_mul(prod, ue, ee)
            t1 = wt("t1")[0:nrow, :]
            v.tensor_mul(t1, inter, ee)
            r0 = wt("r0")[0:nrow, :]
            rc = wt("rc")[0:nrow, :]
            v.reciprocal_approx_accurate(out=rc, in_=prod, scratch=r0)
            gnum = wt("gnum")[0:nrow, :]
            v._custom_dve(ops["DETR_ADDSQ"], out=gnum, in0=t1, in1=ue)
            gterm = wt("gterm")[0:nrow, :]
            g.tensor_mul(gterm, gnum, rc)

            l1t = wt("l1t")[0:nrow, :]
            g.tensor_add(l1t, s01, s23)
            pre = wt("pre")[0:nrow, :]
            g.tensor_sub(pre, l1t, gterm)
            g.tensor_mul(costall[0:nrow, k, :], pre, bcS("mask2"))

            # chunked slab flushes
            if k == NTILES - 1:
                for lo, hi in ((0, 4), (4, 7), (7, 10)):
                    nc.sync.dma_start(out=cost_d[:, lo:hi, :],
                                      in_=costall[:, lo:hi, :])
                for lo, hi in ((0, 7), (7, 12), (12, NSEG)):
                    nc.sync.dma_start(out=nll_d[:, lo:hi, :],
                                      in_=nllall[:, lo:hi, :])

    nc.finalize()
    return nc


def _decomp3(vv):
    """f32 -> three bf16 arrays summing to v (error ~2^-25 |v|)."""
    vv = np.asarray(vv, np.float32)
    h = vv.astype(NPBF)
    r = vv - h.astype(np.float32)
    m = r.astype(NPBF)
    lo = (r - m.astype(np.float32)).astype(NPBF)
    return h, m, lo


def _host_prep(prob_class, predict_bbox, labels):
    pc = np.asarray(prob_class, np.float32)
    pb = np.asarray(predict_bbox, np.float32)
    lab = np.asarray(labels, np.float32)

    lb = lab[..., :4] / IMG_SIZE
    cls = lab[..., 4].astype(np.int32)
    mask = (cls != NO_OBJ).astype(np.float32)

    lcx, lcy, lw, lh = (lb[..., i] for i in range(4))
    half = np.float32(0.5)
    lx0 = lcx - half * lw
    lx1 = lcx + half * lw
    ly0 = lcy - half * lh
    ly1 = lcy + half * lh
    lwE = lx1 - lx0
    lhE = ly1 - ly0
    areal = lwE * lhE

    labvals = {"lx1": lx1, "ly1": ly1, "dcx": lcx,
               "lx0": lx0, "ly0": ly0, "dcy": lcy,
               "dw": lw, "dh": lh, "mask2": 2.0 * mask,
               "uepre": areal, "ewpre": lwE, "ehpre": lhE}
    qlab = np.zeros((B, 1800), np.float32)
    for name, q in SLOT.items():
        off = 450 * (q // 3) + 150 * (q % 3)
        qlab[:, off:off + T] = labvals[name]

    pcx, pcy, pw, ph = (pb[..., i] for i in range(4))
    px0 = pcx - half * pw
    px1 = pcx + half * pw
    py0 = pcy - half * ph
    py1 = pcy + half * ph
    pwE = px1 - px0
    phE = py1 - py0
    apE = pwE * phE + EPS
    presvals = {"npcx": -pcx, "npcy": -pcy, "npw": -pw, "nph": -ph,
                "apE": apE, "pwE": pwE, "phE": phE}   # [B, P] each

    # scal packed [128, NTILES, 4] per core (row r of tile k = global row
    # 128k + r)
    scal_rows = np.stack([px1, px0, py1, py0], axis=-1).reshape(
        NCORES, ROWS, 4)
    scal = np.zeros((NCORES, 128, NTILES, 4), np.float32)
    for k in range(NTILES):
        r0, r1 = k * 128, min(k * 128 + 128, ROWS)
        scal[:, 0:r1 - r0, k, :] = scal_rows[:, r0:r1, :]

    # nll rhs [KN, B-local, T] per core
    rhsnm = np.zeros((B, KN, T), np.float32)
    bi = np.repeat(np.arange(B), T)
    ti = np.tile(np.arange(T), B)
    rhsnm[bi, cls.ravel(), ti] = np.float32(-1.0)
    rhsnm[:, 81, :] = np.float32(4.0) * mask
    rhs_t = np.ascontiguousarray(
        rhsnm.reshape(NCORES, BL, KN, T).transpose(0, 2, 1, 3))

    # ln arg [C, B-local, P] per core
    pt_t = np.ascontiguousarray(
        pc.reshape(NCORES, BL, P, C).transpose(0, 3, 1, 2))

    qlab3 = _decomp3(qlab)                       # 3 x [B, 1800]
    pres3 = {n: _decomp3(val) for n, val in presvals.items()}

    bcr = np.zeros((NCORES, NTILES, NK, 2312), NPBF)
    for k, segs in enumerate(TILES):
        for core in range(NCORES):
            for si, (b, p0, ln, off) in enumerate(segs):
                gb = core * BL + b
                for p3 in range(3):
                    # label rows: values on rhs cols, sel on lhsT cols
                    bcr[core, k, 3 * si + p3, 0:1800] = qlab3[p3][gb]
                    for j in range(4):
                        bcr[core, k, 3 * si + p3,
                            1800 + 128 * j + off:1800 + 128 * j + off + ln] \
                            = NPBF(1.0)
            # presum rows 6..14: per-bank slot m
            for name, q in SLOT.items():
                pn = PRESUM_OF.get(name)
                if pn is None:
                    continue
                j, m = q // 3, q % 3
                coff = 450 * j + 150 * m
                for si, (b, p0, ln, off) in enumerate(segs):
                    gb = core * BL + b
                    for p3 in range(3):
                        row = 6 + 3 * m + p3
                        bcr[core, k, row,
                            1800 + 128 * j + off:1800 + 128 * j + off + ln] \
                            = pres3[pn][p3][gb, p0:p0 + ln]
                        bcr[core, k, row, coff:coff + T] = NPBF(1.0)

    in_maps = []
    for core in range(NCORES):
        in_maps.append({
            "pt": pt_t[core],
            "rhsnm": rhs_t[core],
            "bcr": bcr[core],
            "scal": scal[core],
        })
    return in_maps


def _hungarian_np(cost):
    """Jonker-Volgenant LSA fallback (same algorithm as scipy)."""
    cost = np.asarray(cost, dtype=np.float64)
    n, m = cost.shape
    INF = float("inf")
    u = np.zeros(n + 1)
    vv = np.zeros(m + 1)
    p = np.zeros(m + 1, dtype=np.int64)
    way = np.zeros(m + 1, dtype=np.int64)
    for i in range(1, n + 1):
        p[0] = i
        j0 = 0
        minv = np.full(m + 1, INF)
        used = np.zeros(m + 1, dtype=bool)
        while True:
            used[j0] = True
            i0 = p[j0]
            free = ~used[1:]
            cur = cost[i0 - 1] - u[i0] - vv[1:]
            better = free & (cur < minv[1:])
            minv[1:] = np.where(better, cur, minv[1:])
            way[1:] = np.where(better, j0, way[1:])
            masked = np.where(free, minv[1:], INF)
            j1 = int(np.argmin(masked)) + 1
            delta = masked[j1 - 1]
            uj = np.nonzero(used)[0]
            u[p[uj]] += delta
            vv[uj] -= delta
            minv[1:] = np.where(free, minv[1:] - delta, minv[1:])
            j0 = j1
            if p[j0] == 0:
                break
        while j0 != 0:
            j1 = way[j0]
            p[j0] = p[j1]
            j0 = j1
    cols = np.nonzero(p[1:])[0]
    rows = p[1:][cols] - 1
    order = np.argsort(rows)
    return rows[order], cols[order]


def _lsa(cost):
    try:
        from scipy.optimize import linear_sum_assignment
        return linear_sum_assignment(cost)
    except Exception:
        return _hungarian_np(cost)


def _install_profile_shim():
    import types

    if "antenv.axon_hooks" not in sys.modules:
        import antenv

        mod = types.ModuleType("antenv.axon_hooks")
        mod._HOOK = None
        mod.set_axon_ntff_profile_hook = lambda h: setattr(mod, "_HOOK", h)
        mod.get_axon_ntff_profile_hook = lambda: mod._HOOK
        sys.modules["antenv.axon_hooks"] = mod
        antenv.axon_hooks = mod
    import antenv.axon_hooks as ah

    if ah.get_axon_ntff_profile_hook() is None:
        try:
            from trn_agent_boot.trn_boot import _ntff_profile_via_ctypes

            hook = _ntff_profile_via_ctypes("/opt/axon/libaxon_pjrt.so")
            if hook is not None:
                ah.set_axon_ntff_profile_hook(hook)
        except Exception:
            pass
    import concourse.bass_utils as bu

    bu.upload_artifacts = lambda tmpdir: f"local:{tmpdir}"


def kernel(prob_class, predict_bbox, labels):
    global LAST_EXEC_NS, LAST_RESULTS
    if "nc" not in _CACHE:
        _CACHE["nc"] = _build_program()
    nc = _CACHE["nc"]

    in_maps = _host_prep(prob_class, predict_bbox, labels)
    if TRACE:
        _install_profile_shim()
    res = run_bass_kernel_spmd(nc, in_maps, list(range(NCORES)), trace=TRACE)
    LAST_EXEC_NS = res.exec_time_ns
    LAST_RESULTS = res

    cost = np.empty((B, P, T), np.float32)
    for core in range(NCORES):
        cst = res.results[core]["cost"]    # [128, NTILES, T]
        nll = res.results[core]["nllm"]    # [128, NSEG, T]
        slab = np.empty((ROWS, T), np.float32)
        for k in range(NTILES):
            r0, r1 = k * 128, min(k * 128 + 128, ROWS)
            slab[r0:r1] = cst[0:r1 - r0, k, :]
        for sidx, (k, si, b, p0, ln, off) in enumerate(SEGS):
            r0 = k * 128 + off
            slab[r0:r0 + ln] += nll[0:ln, sidx, :]
        cost[core * BL:(core + 1) * BL] = slab.reshape(BL, P, T)

    loss = np.float32(0.0)
    cost64 = cost.astype(np.float64)
    for i in range(B):
        r, c = _lsa(cost64[i])
        loss = loss + cost[:, r, c].mean(dtype=np.float32)
    return np.float32(loss)
